# revision 1
# baseline (speedup 1.0000x reference)
"""Expected Calibration Error (histogram binning) on 8 Trainium2 NeuronCores.

kernel(outputs [1e6,100] f32, targets [1e6] int) -> f32 scalar, matching the
reference softmax/argmax/10-bin ECE. Data-parallel over the batch; each core
streams its 50 MB shard once from HBM (~140 us/core roofline at 358 GB/s).

Layout trick: every per-row ECE statistic (row max, row sum of exp, true-class
prob, argmax==target, bin membership) is invariant to a cyclic rotation of the
row's class axis. The host rolls each row left by its target class — the
true-class logit lands in column 0 for every row — and packs rows densely onto
a [8 cores, 128 partitions, 980 rows] grid (0.35% padding; pad rows are
[-300, 0, ...], whose exp underflows to exactly 0 so p == 0 and the
reference's own p > 0 rule excludes them). Rows are dealt sorted-by-class only
to make the host-side rolling two contiguous slice copies per class.

Device, per chunk of 49 rows/partition ([128, 49, 100] tile):
  - DVE:  segmented reduce_max over classes (for argmax-equality correctness)
  - ACT:  exp of the whole tile; strided copy of column 0 (true-class logit)
  - PE:   row-sum-of-exp for 79 classes via accumulating identity matmuls
  - DVE:  segmented reduce_add for the remaining 21 classes (engine balance)
Finish (two interleaved slabs): p = exp(x_t) * (1/s) (DVE reciprocal),
correct = (x_t == m), z = p * correct (GPSIMD), then per boundary b of
linspace(0,1,11), fused full-tile-scan + per-partition accumulate ops:
  C_b = count(p > b)          (DVE  is_gt + accumulate)
  R_b = sum(relu(p - b))      (ACT  relu with bias=-b + accumulate)
  Z_b = count(z > b)          (DVE  is_gt + accumulate)
Host: sum the 8x128 partials in f64; SP_b = R_b + b*C_b recovers the
cumulative sum of confidences; adjacent differences give the 10 bins; finish
the ECE scalar exactly as the reference does. All four engines plus DMA land
within ~10% of the per-core HBM roofline (cost model: ~170 us vs ~140 us DMA).
"""

import os
import sys
import tempfile

import numpy as np

if "/opt/trn_rl_repo" not in sys.path:
    sys.path.insert(0, "/opt/trn_rl_repo")

# Persistent jax/PJRT executable cache (includes the compiled NEFF): makes
# repeat invocations and the subprocess-retry path skip the ~60s neuronx
# compile. Must be set before jax initializes.
os.environ.setdefault(
    "JAX_COMPILATION_CACHE_DIR",
    os.path.join(tempfile.gettempdir(), "jaxcache"),
)

N = 1_000_000
C = 100
NCORES = 8
P = 128
W = 980
G = 49
CHUNKS = W // G      # 20
PECLS = int(os.environ.get("KV_PECLS", "79"))
_SLAB_ENDS = [int(v) for v in os.environ.get("KV_SLABS", "15,20").split(",")]
NSLAB = len(_SLAB_ENDS)
XBUFS = int(os.environ.get("KV_XBUFS", "3"))
EBUFS = int(os.environ.get("KV_EBUFS", "2"))
NPAD = NCORES * P * W
SENT = -300.0

_BOUNDS = np.linspace(0.0, 1.0, 11).astype(np.float32)

_built = {}


def _build_program():
    if "nc" in _built:
        return _built["nc"]

    import concourse.bacc as bacc
    import concourse.tile as tile
    from concourse import mybir

    f32 = mybir.dt.float32
    Alu = mybir.AluOpType
    Act = mybir.ActivationFunctionType
    AxX = mybir.AxisListType.X

    nc = bacc.Bacc("TRN2", target_bir_lowering=False, debug=False)
    x_d = nc.dram_tensor("x", [P, W * C], f32, kind="ExternalInput").ap()
    ident_d = nc.dram_tensor("ident", [P, P], f32, kind="ExternalInput").ap()
    nbnd_d = nc.dram_tensor("nbnd", [P, 11], f32, kind="ExternalInput").ap()
    acc_d = nc.dram_tensor("acc", [P, 33 * NSLAB], f32, kind="ExternalOutput").ap()

    slab_cols = [0] + [e * G for e in _SLAB_ENDS]
    assert slab_cols[-1] == W
    MAXSLAB = max(b - a for a, b in zip(slab_cols, slab_cols[1:]))

    with tile.TileContext(nc) as tc:
        with (
            tc.tile_pool(name="consts", bufs=1) as consts,
            tc.tile_pool(name="stats", bufs=1) as stats,
            tc.tile_pool(name="xin", bufs=XBUFS) as xin,
            tc.tile_pool(name="etmp", bufs=EBUFS) as etmp,
            tc.tile_pool(name="psum", bufs=2, space="PSUM") as psp,
        ):
            ident_t = consts.tile([P, P], f32)
            nc.gpsimd.dma_start(ident_t[:], ident_d[:, :])
            nbnd_t = consts.tile([P, 11], f32)
            nc.gpsimd.dma_start(nbnd_t[:], nbnd_d[:, :])

            M = stats.tile([P, W], f32, tag="M")
            S = stats.tile([P, W], f32, tag="S")
            SD = stats.tile([P, W], f32, tag="SD")
            XT = stats.tile([P, W], f32, tag="XT")
            corr = stats.tile([P, W], f32, tag="corr")
            ET = stats.tile([P, W], f32, tag="ET")
            RS = stats.tile([P, W], f32, tag="RS")
            PT = stats.tile([P, W], f32, tag="PT")
            Z = stats.tile([P, W], f32, tag="Z")
            ACC = stats.tile([P, 33 * NSLAB], f32, tag="ACC")
            junkW = stats.tile([P, MAXSLAB], f32, tag="junkW")
            junkR = stats.tile([P, MAXSLAB], f32, tag="junkR")

            def finish_slab(si):
                c0, c1 = slab_cols[si], slab_cols[si + 1]
                nc.vector.tensor_tensor(
                    corr[:, c0:c1], XT[:, c0:c1], M[:, c0:c1], op=Alu.is_equal
                )
                nc.scalar.activation(ET[:, c0:c1], XT[:, c0:c1], Act.Exp)
                nc.gpsimd.tensor_tensor(
                    S[:, c0:c1], S[:, c0:c1], SD[:, c0:c1], op=Alu.add
                )
                nc.vector.reciprocal(RS[:, c0:c1], S[:, c0:c1])
                nc.vector.tensor_tensor(
                    PT[:, c0:c1], ET[:, c0:c1], RS[:, c0:c1], op=Alu.mult
                )
                nc.gpsimd.tensor_tensor(
                    Z[:, c0:c1], PT[:, c0:c1], corr[:, c0:c1], op=Alu.mult
                )
                ab = 33 * si
                for b in range(11):
                    lo = float(_BOUNDS[b])
                    nw = c1 - c0
                    nc.vector.tensor_scalar(
                        junkW[:, :nw], PT[:, c0:c1], lo, None,
                        op0=Alu.is_gt, op1=Alu.add,
                        accum_out=ACC[:, ab + b:ab + b + 1],
                    )
                    nc.scalar.activation(
                        junkR[:, :nw], PT[:, c0:c1], Act.Relu,
                        bias=nbnd_t[:, b:b + 1],
                        accum_out=ACC[:, ab + 11 + b:ab + 12 + b],
                    )
                    nc.vector.tensor_scalar(
                        junkW[:, :nw], Z[:, c0:c1], lo, None,
                        op0=Alu.is_gt, op1=Alu.add,
                        accum_out=ACC[:, ab + 22 + b:ab + 23 + b],
                    )

            for k in range(CHUNKS):
                X = xin.tile([P, G * C], f32)
                nc.sync.dma_start(X[:], x_d[:, k * G * C:(k + 1) * G * C])
                x3 = X[:].rearrange("p (g c) -> p g c", c=C)
                nc.vector.tensor_reduce(
                    M[:, k * G:(k + 1) * G], x3, axis=AxX, op=Alu.max
                )
                nc.scalar.copy(
                    XT[:, k * G:(k + 1) * G],
                    x3[:, :, 0:1].rearrange("p g c -> p (g c)"),
                )
                E = etmp.tile([P, G * C], f32)
                nc.scalar.activation(E[:], X[:], Act.Exp)
                e3 = E[:].rearrange("p (g c) -> p g c", c=C)
                PS = psp.tile([P, G], f32)
                for cc in range(PECLS):
                    nc.tensor.matmul(
                        PS[:], ident_t[:],
                        e3[:, :, cc:cc + 1].rearrange("p g c -> p (g c)"),
                        start=(cc == 0), stop=(cc == PECLS - 1),
                    )
                nc.scalar.copy(S[:, k * G:(k + 1) * G], PS[:])
                nc.vector.tensor_reduce(
                    SD[:, k * G:(k + 1) * G], e3[:, :, PECLS:C],
                    axis=AxX, op=Alu.add,
                )
                if (k + 1) in _SLAB_ENDS:
                    finish_slab(_SLAB_ENDS.index(k + 1))

            nc.sync.dma_start(acc_d[:, :], ACC[:])

    nc.compile()
    _built["nc"] = nc
    return nc


def _prep_inputs(outputs, targets):
    """Sort rows by class, roll each row left by its class, pack densely."""
    x = np.ascontiguousarray(np.asarray(outputs, dtype=np.float32))
    t = np.asarray(targets).astype(np.int64).ravel()
    order = np.argsort(t, kind="stable")
    cnt = np.bincount(t, minlength=C)
    starts = np.zeros(C + 1, np.int64)
    starts[1:] = np.cumsum(cnt)

    Xr = np.empty((NPAD, C), np.float32)
    for c in range(C):
        s0, s1 = starts[c], starts[c + 1]
        if s1 == s0:
            continue
        src = x[order[s0:s1]]
        Xr[s0:s1, :C - c] = src[:, c:]
        if c:
            Xr[s0:s1, C - c:] = src[:, :c]
    Xr[N:] = 0.0
    Xr[N:, 0] = SENT

    Xv = Xr.reshape(NCORES, P, W * C)
    ident = np.eye(P, dtype=np.float32)
    nbnd = np.broadcast_to(-_BOUNDS, (P, 11)).copy()
    return [{"x": Xv[c], "ident": ident, "nbnd": nbnd} for c in range(NCORES)]


def _postprocess(acc_list):
    A = np.stack(acc_list)
    tot = A.astype(np.float64).sum(axis=(0, 1))
    tot = tot.reshape(NSLAB, 33).sum(axis=0)
    Cg, R, Zg = tot[0:11], tot[11:22], tot[22:33]
    bounds = _BOUNDS.astype(np.float64)
    SPcum = R + bounds * Cg                  # sum of p over {p > bound[b]}
    cnt = Cg[:10] - Cg[1:]
    sp = SPcum[:10] - SPcum[1:]
    sc = Zg[:10] - Zg[1:]
    nonempty = cnt > 0
    denom = np.where(nonempty, cnt, 1.0)
    ece = np.sum(np.where(nonempty, cnt * np.abs(sp / denom - sc / denom), 0.0))
    total = cnt.sum()
    val = ece / max(total, 1.0) if total > 0 else 0.0
    return np.float32(val)


def _exec(in_maps, trace=False):
    from concourse.bass_utils import run_bass_kernel_spmd

    nc = _build_program()
    res = run_bass_kernel_spmd(
        nc, in_maps, core_ids=list(range(NCORES)), trace=trace
    )
    return [res.results[c]["acc"] for c in range(NCORES)], res


def _subrun(tmpdir):
    """Subprocess entry: load prepped inputs, execute, save partials."""
    in_maps = []
    for c in range(NCORES):
        in_maps.append({
            "x": np.load(f"{tmpdir}/x{c}.npy"),
            "ident": np.load(f"{tmpdir}/ident.npy"),
            "nbnd": np.load(f"{tmpdir}/nbnd.npy"),
        })
    accs, _ = _exec(in_maps)
    np.save(f"{tmpdir}/accs.npy", np.stack(accs))


def _exec_subprocess(in_maps):
    """Run the device step in a fresh process (fresh PJRT client) — recovers
    from transient 'accelerator device unrecoverable' states."""
    import subprocess
    import tempfile

    here = os.path.dirname(os.path.abspath(__file__))
    with tempfile.TemporaryDirectory() as td:
        for c in range(NCORES):
            np.save(f"{td}/x{c}.npy", in_maps[c]["x"])
        np.save(f"{td}/ident.npy", in_maps[0]["ident"])
        np.save(f"{td}/nbnd.npy", in_maps[0]["nbnd"])
        code = (
            f"import sys; sys.path.insert(0, {here!r}); "
            f"import kernel; kernel._subrun({td!r})"
        )
        subprocess.run([sys.executable, "-c", code], check=True, timeout=2400)
        accs = np.load(f"{td}/accs.npy")
    return [accs[c] for c in range(NCORES)]


def _run(outputs, targets, trace=False):
    import time

    in_maps = _prep_inputs(outputs, targets)
    accs = None
    last_err = None
    try:
        accs, res = _exec(in_maps, trace=trace)
    except Exception as e:  # transient device-unrecoverable errors
        last_err = e
        res = None
        sys.stderr.write(f"kernel: in-process exec failed: {e}\n")
    if accs is None:
        for attempt in range(3):
            try:
                time.sleep(5.0)
                accs = _exec_subprocess(in_maps)
                break
            except Exception as e:
                last_err = e
                sys.stderr.write(
                    f"kernel: subprocess exec attempt {attempt} failed: {e}\n"
                )
        else:
            raise last_err
    val = _postprocess(accs)
    return val, res


def kernel(outputs, targets):
    val, _ = _run(outputs, targets, trace=False)
    return val

